# revision 15
# baseline (speedup 1.0000x reference)
"""Bahdanau additive attention on 8 TRN2 NeuronCores (data-parallel over batch).

Per batch item b (T=2048 steps, DH=DZ=DA=1024):
    pre[a, t] = sum_d Wh[a, d] h[t, d] + bias_b[a]     (PE, bf16)
                bias_b = Wz @ z_b + b                   (PE, once for all b)
    k         = tanh(pre)                               (ACT, bias fused)
    score[t]  = sum_a v[a] k[a, t]                      (PE matvec)
    e         = exp(score)  (unnormalized; |score| <= sum|v| ~ 23)
    C_raw[d]  = sum_t e[t] h[t, d]                      (DVE mul + reduce)
    C         = C_raw / sum_t e[t]                      (normalized at the end)

Data movement: h enters the tensor engine with d on partitions (hT).
f32 has no XBAR transpose, so h is cast f32->bf16 by a gpsimd DMA
(DRAM->DRAM, cast in flight - no SBUF staging, no DVE pass), then read
back through the DMA XBAR transpose into a chunk-major [128, c, ko, 512]
SBUF layout (contiguous transpose destinations). W/z take the same path.
Transposes are issued from the sync engine so the scalar engine stays
free for tanh/exp.

e is broadcast across partitions with a K=1 ones matmul into PSUM
(no DRAM round trip), copied to bf16 SBUF by ACT, and folded into
C partials by one DVE multiply per chunk (ko-broadcast AP) + 8 reduces.
Normalization happens once at the very end for all 8 batches: one
reciprocal, one f32 ones-matmul broadcast, one elementwise scale, one
f32 PE transpose, and a single contiguous store.

Sharding: batch 64 -> 8 cores x 8 batches. W/b/v replicated. No
collectives; the host concatenates the per-core [8, 1024] outputs.
"""

import os
import sys

for _p in ("/opt/trn_rl_repo", "/root/.axon_site/_ro/trn_rl_repo"):
    if os.path.isdir(_p) and _p not in sys.path:
        sys.path.insert(0, _p)

import numpy as np

P = 128
NCORES = 8
B, T, D = 64, 2048, 1024
BL = B // NCORES          # local batches per core
DO = D // P               # 8 128-blocks in D (d, a, dz alike)
CH = 512                  # matmul t-chunk (= one PSUM bank)
NCH = T // CH

_graph_cache = None


def _build(finalize=True):
    import concourse.bass as bass  # noqa: F401
    import concourse.tile as tile
    import concourse.mybir as mybir
    from concourse import bacc
    from concourse.masks import make_identity

    F32 = mybir.dt.float32
    BF16 = mybir.dt.bfloat16
    Tanh = mybir.ActivationFunctionType.Tanh
    Exp = mybir.ActivationFunctionType.Exp
    Add = mybir.AluOpType.add

    nc = bacc.Bacc(None, target_bir_lowering=False)
    Z_ext = nc.declare_dram_parameter("Z_st", [BL, D], F32, isOutput=False)
    h_ext = nc.declare_dram_parameter("h_n_state", [BL, T, D], F32, isOutput=False)
    W_ext = nc.declare_dram_parameter("W", [D, 2 * D], F32, isOutput=False)
    b_ext = nc.declare_dram_parameter("b", [D], F32, isOutput=False)
    v_ext = nc.declare_dram_parameter("v", [D], F32, isOutput=False)
    out_ext = nc.declare_dram_parameter("out", [BL, D], F32, isOutput=True)

    with tile.TileContext(nc) as tc:
        with (
            tc.tile_pool(name="const", bufs=1) as const_pool,
            tc.tile_pool(name="hT", bufs=3) as hT_pool,
            tc.tile_pool(name="kc", bufs=2) as kc_pool,
            tc.tile_pool(name="ebc_sb", bufs=2) as ebc_sb_pool,
            tc.tile_pool(name="tmp", bufs=2) as tmp_pool,
            tc.tile_pool(name="misc", bufs=2) as misc_pool,
            tc.tile_pool(name="dram", bufs=1, space="DRAM") as dram_pool,
            tc.tile_pool(name="hbf", bufs=2, space="DRAM") as hbf_pool,
            tc.tile_pool(name="ppre", bufs=4, space="PSUM") as ppre_pool,
            tc.tile_pool(name="pscore", bufs=1, space="PSUM") as pscore_pool,
            tc.tile_pool(name="pebc", bufs=2, space="PSUM") as pebc_pool,
            tc.tile_pool(name="pmisc", bufs=1, space="PSUM") as pmisc_pool,
        ):
            # ---------------- weights / constants prep ----------------
            # cast f32 -> bf16 in-flight, DRAM -> DRAM (gpsimd SWDGE).
            # SWDGE assigns each DMA instruction to one of 8 queues that
            # fair-share HBM, so the W path is split into 128-row blocks
            # (one per a-block) interleaved Wh/Wz: WhT/WzT a-slices land
            # progressively and the first matmul + bias groups unblock
            # long before the full W is across.
            Wh_bf = dram_pool.tile([D, D], BF16, tag="wh_bf")
            Wz_bf = dram_pool.tile([D, D], BF16, tag="wz_bf")
            z_bf = dram_pool.tile([16, D], BF16, tag="z_bf")
            nc.gpsimd.dma_start(out=z_bf[0:BL, :], in_=Z_ext[:, :])
            for ao in range(DO):
                rsl = slice(ao * P, (ao + 1) * P)
                nc.gpsimd.dma_start(out=Wh_bf[rsl, :], in_=W_ext[rsl, 0:D])
                nc.gpsimd.dma_start(out=Wz_bf[rsl, :], in_=W_ext[rsl, D : 2 * D])

            # XBAR transposes all on the sync engine: exactly one engine
            # may drive the xbar at a time (concurrent transposes from two
            # HWDGE engines raced and corrupted data), and keeping its
            # queues transpose-only avoids xbar-mode transitions.
            # WhT[p, ko, a] = W[a, ko*128 + p] (d half); WzT the dz half.
            WzT = const_pool.tile([P, DO, D], BF16, tag="WzT")
            WhT = const_pool.tile([P, DO, D], BF16, tag="WhT")
            zT = const_pool.tile([P, DO, 16], BF16, tag="zT")
            nc.sync.dma_start_transpose(zT, z_bf[:, :])
            for ao in range(DO):
                rsl = slice(ao * P, (ao + 1) * P)
                nc.sync.dma_start_transpose(WhT[:, :, rsl], Wh_bf[rsl, :])
                nc.sync.dma_start_transpose(WzT[:, :, rsl], Wz_bf[rsl, :])

            # v in [a_p, a_o] column layout; b as a bf16 row (folded into
            # the biasvec matmul as a K=1 rank-1 update)
            vstg = const_pool.tile([P, DO], F32, tag="vstg")
            for ao in range(DO):
                nc.sync.dma_start(
                    out=vstg[:, ao : ao + 1],
                    in_=v_ext[ao * P : (ao + 1) * P].rearrange("(p o) -> p o", o=1),
                )
            v_colT = const_pool.tile([P, DO], BF16, tag="v_colT")
            nc.vector.tensor_copy(out=v_colT, in_=vstg)

            b_row = const_pool.tile([1, D], F32, tag="b_row")
            nc.sync.dma_start(out=b_row, in_=b_ext.rearrange("(o d) -> o d", o=1))
            b_bf = const_pool.tile([1, D], BF16, tag="b_bf")
            nc.vector.tensor_copy(out=b_bf, in_=b_row)

            ones_bf = const_pool.tile([1, P], BF16, tag="ones_bf")
            nc.vector.memset(ones_bf, 1.0)
            ones_f32 = const_pool.tile([1, P], F32, tag="ones_f32")
            nc.vector.memset(ones_f32, 1.0)
            ident = const_pool.tile([P, P], F32, tag="ident")
            make_identity(nc, ident)

            # biasvec[a, b] = sum_dz Wz[a, dz] z[b, dz] + b[a]
            pvec = pmisc_pool.tile([P, DO, BL], F32, tag="pmisc")
            for ao in range(DO):
                aosl = slice(ao * P, (ao + 1) * P)
                for ko in range(DO):
                    nc.tensor.matmul(
                        pvec[:, ao, :],
                        lhsT=WzT[:, ko, aosl],
                        rhs=zT[:, ko, :BL],
                        start=(ko == 0),
                        stop=False,
                    )
                nc.tensor.matmul(
                    pvec[:, ao, :],
                    lhsT=b_bf[0:1, aosl],
                    rhs=ones_bf[0:1, :BL],
                    start=False,
                    stop=True,
                )
            biasvec = const_pool.tile([P, DO, BL], F32, tag="biasvec")
            for ao in range(DO):
                nc.vector.tensor_copy(out=biasvec[:, ao], in_=pvec[:, ao])

            # accumulators shared across batches
            sums_all = const_pool.tile([1, BL, NCH], F32, tag="sums_all")
            CT_all = const_pool.tile([P, BL, DO], F32, tag="CT_all")

            # ---------------- per-batch pipeline ----------------
            for b in range(BL):
                # cast f32->bf16 in-flight (gpsimd DMA, DRAM->DRAM) then
                # XBAR transpose, quadrant at a time (chunk-major dest)
                hbf = hbf_pool.tile([T, D], BF16, tag="hbf")
                hT = hT_pool.tile([P, NCH, DO, CH], BF16, tag="hT")
                for c in range(NCH):
                    rsl = slice(c * CH, (c + 1) * CH)
                    nc.gpsimd.dma_start(out=hbf[rsl, :], in_=h_ext[b, rsl, :])
                    nc.sync.dma_start_transpose(hT[:, c], hbf[rsl, :])

                eraw = misc_pool.tile([1, T], BF16, tag="eraw")
                CTpart = misc_pool.tile([P, DO, NCH], F32, tag="CTpart")
                for c in range(NCH):
                    csl = slice(c * CH, (c + 1) * CH)
                    # pre = Wh @ hT (+bias) -> k = tanh
                    kc = kc_pool.tile([P, DO, CH], BF16, tag="kc")
                    for ao in range(DO):
                        pre = ppre_pool.tile([P, CH], F32, tag="pre")
                        for ko in range(DO):
                            nc.tensor.matmul(
                                pre,
                                lhsT=WhT[:, ko, ao * P : (ao + 1) * P],
                                rhs=hT[:, c, ko],
                                start=(ko == 0),
                                stop=(ko == DO - 1),
                            )
                        nc.scalar.activation(
                            out=kc[:, ao, :], in_=pre, func=Tanh,
                            bias=biasvec[:, ao, b : b + 1],
                        )
                    # score = v . k  (PE matvec)
                    psc = pscore_pool.tile([1, CH], F32, tag="psc")
                    for ao in range(DO):
                        nc.tensor.matmul(
                            psc,
                            lhsT=v_colT[:, ao : ao + 1],
                            rhs=kc[:, ao],
                            start=(ao == 0),
                            stop=(ao == DO - 1),
                        )
                    # e = exp(score) (bf16) + chunk sum (f32)
                    nc.scalar.activation(
                        out=eraw[:, csl], in_=psc, func=Exp,
                        accum_out=sums_all[0:1, b, c : c + 1],
                    )
                    # broadcast e across partitions via K=1 ones matmul
                    ebc = pebc_pool.tile([P, CH], F32, tag="ebc")
                    nc.tensor.matmul(
                        ebc, lhsT=ones_bf[0:1, :], rhs=eraw[0:1, csl],
                        start=True, stop=True,
                    )
                    ebc_sb = ebc_sb_pool.tile([P, CH], BF16, tag="ebc_sb")
                    nc.scalar.copy(out=ebc_sb, in_=ebc)
                    # C partials: one mul over all ko, then per-ko reduce
                    tmp = tmp_pool.tile([P, DO, CH], BF16, tag="tmp")
                    nc.vector.tensor_mul(
                        out=tmp, in0=hT[:, c],
                        in1=ebc_sb[:, None, :].to_broadcast((P, DO, CH)),
                    )
                    for ko in range(DO):
                        nc.vector.tensor_reduce(
                            out=CTpart[:, ko, c : c + 1], in_=tmp[:, ko],
                            axis=mybir.AxisListType.X, op=Add,
                        )
                # fold the 4 chunk partials
                nc.vector.tensor_reduce(
                    out=CT_all[:, b, :], in_=CTpart,
                    axis=mybir.AxisListType.X, op=Add,
                )

            # ---------------- finalize all batches ----------------
            ssum = const_pool.tile([1, BL], F32, tag="ssum")
            nc.vector.tensor_reduce(
                out=ssum, in_=sums_all, axis=mybir.AxisListType.X, op=Add
            )
            inv = const_pool.tile([1, BL], F32, tag="inv")
            nc.vector.reciprocal(out=inv, in_=ssum)
            pinv = pmisc_pool.tile([P, BL], F32, tag="pmisc")
            nc.tensor.matmul(
                pinv, lhsT=ones_f32[0:1, :], rhs=inv[0:1, :], start=True, stop=True
            )
            nc.vector.tensor_mul(
                out=CT_all, in0=CT_all,
                in1=pinv[:, :, None].to_broadcast((P, BL, DO)),
            )
            # transpose [d_p, (b ko)] -> [(b ko), d_p] and store contiguously
            ptr = pmisc_pool.tile([BL * DO, P], F32, tag="pmisc")
            nc.tensor.transpose(
                ptr, CT_all.rearrange("p b k -> p (b k)"), ident
            )
            tr_sb = const_pool.tile([BL * DO, P], F32, tag="tr_sb")
            nc.vector.tensor_copy(out=tr_sb, in_=ptr)
            nc.sync.dma_start(
                out=out_ext.rearrange("b (k p) -> (b k) p", p=P), in_=tr_sb
            )

    if finalize:
        nc.finalize()
    return nc


def _get_graph():
    global _graph_cache
    if _graph_cache is None:
        _graph_cache = _build()
    return _graph_cache


def kernel(Z_st, h_n_state, W, b, v, _trace=False):
    from concourse.bass_utils import run_bass_kernel_spmd

    nc = _get_graph()
    Z_st = np.ascontiguousarray(np.asarray(Z_st, dtype=np.float32))
    h_n_state = np.ascontiguousarray(np.asarray(h_n_state, dtype=np.float32))
    W = np.ascontiguousarray(np.asarray(W, dtype=np.float32))
    b = np.ascontiguousarray(np.asarray(b, dtype=np.float32))
    v = np.ascontiguousarray(np.asarray(v, dtype=np.float32))

    in_maps = []
    for c in range(NCORES):
        sl = slice(c * BL, (c + 1) * BL)
        in_maps.append(
            {
                "Z_st": Z_st[sl],
                "h_n_state": h_n_state[sl],
                "W": W,
                "b": b,
                "v": v,
            }
        )
    res = run_bass_kernel_spmd(nc, in_maps, core_ids=list(range(NCORES)), trace=_trace)
    out = np.concatenate([res.results[c]["out"] for c in range(NCORES)], axis=0)
    if _trace:
        kernel.last_exec_time_ns = res.exec_time_ns
        kernel.last_results = res
    return out
